# revision 1
# baseline (speedup 1.0000x reference)
"""AttnBlock (GroupNorm -> QKV -> 4096x4096 spatial attention -> proj -> residual)
for Trainium2, sharded over 8 NeuronCores.

Sharding: core = (batch b, query-slice s); b = core//4, s = core%4.
Each core computes K/V for its full batch image (redundant across the 4 cores
of a batch) and attention/projection for its 1024-query slice. No collectives.

Host-side input prep (exact, tiny): weight transposes, bias folding
(bo2 = bo + wo@bv), and the GroupNorm per-channel affine A = gamma*rstd,
B = beta - mean*A (per batch) so the device applies GroupNorm as one
fused scale+shift while streaming x.

Device layouts (per core):
  hn, q, k: [c, i] with c on partitions (4 chunks of 128)
  vT:       [j, c] with j on partitions (16 tiles of [128, 512] per half)
  scores^T: [j, i] -> softmax along partition axis j:
            exp via ACT (no max subtraction; |scores| <= ~6 by construction),
            denominator via ones-vector matmul, applied after the output
            projection (division commutes with the channel contraction).
All matmuls run as float32r (tf32-like, full PE rate at N=512).
"""
import numpy as np
import concourse.bacc as bacc
import concourse.bass as bass
import concourse.tile as tile
import concourse.mybir as mybir
from concourse.bass_utils import run_bass_kernel_spmd

F32 = mybir.dt.float32
F32R = mybir.dt.float32r
AF = mybir.ActivationFunctionType
OP = mybir.AluOpType

B, C, H, W = 2, 512, 64, 64
HW = H * W                    # 4096
NCORES = 8
NSLICE = 4                    # query slices per batch
SL = HW // NSLICE             # 1024 query positions per core
NG = 32                       # groups
EPS = 1e-6
CCH = C // 128                # 4 channel chunks
NHALF = 2                     # j halves
JH = HW // NHALF              # 2048 j per half
JB = JH // 512                # 4 j-blocks of 512 per half
JC = JH // 128                # 16 j-chunks of 128 per half
IB = SL // 512                # 2 i-blocks of 512
SCALE = float(C) ** -0.5


def build(reps: int = 1):
    nc = bacc.Bacc("TRN2", target_bir_lowering=False)
    dr = {}
    dr["xf"] = nc.dram_tensor("xf", [C, HW], F32, kind="ExternalInput")
    dr["xs"] = nc.dram_tensor("xs", [C, SL], F32, kind="ExternalInput")
    dr["wqT"] = nc.dram_tensor("wqT", [C, C], F32, kind="ExternalInput")
    dr["wkT"] = nc.dram_tensor("wkT", [C, C], F32, kind="ExternalInput")
    dr["wvT"] = nc.dram_tensor("wvT", [C, C], F32, kind="ExternalInput")
    dr["woT"] = nc.dram_tensor("woT", [C, C], F32, kind="ExternalInput")
    # packed per-channel vectors: ball[p, c*5+k], k in {bq, bk, bo2, A, B}
    dr["ball"] = nc.dram_tensor("ball", [128, CCH * 5], F32, kind="ExternalInput")
    dr["xsTb"] = nc.dram_tensor("xsTb", [SL, C], F32, kind="ExternalInput")
    dr["y"] = nc.dram_tensor("y", [SL, C], F32, kind="ExternalOutput")

    with tile.TileContext(nc) as tc:
        _body(nc, tc, reps, dr)
    nc.finalize()
    return nc


def _body(nc, tc, reps, dr):
    from contextlib import ExitStack
    with ExitStack() as ctx:
        pw = ctx.enter_context(tc.tile_pool(name="pw", bufs=1))
        pc = ctx.enter_context(tc.tile_pool(name="pc", bufs=1))
        pq = ctx.enter_context(tc.tile_pool(name="pq", bufs=1))
        pio = ctx.enter_context(tc.tile_pool(name="pio", bufs=1))
        pdr = ctx.enter_context(tc.tile_pool(name="pdr", bufs=2, space="DRAM"))
        pmm = ctx.enter_context(tc.tile_pool(name="pmm", bufs=3, space="PSUM"))
        patt = ctx.enter_context(tc.tile_pool(name="patt", bufs=1, space="PSUM"))

        ball_t = pc.tile([128, CCH * 5], F32, tag="ball", name="ball")
        nc.sync.dma_start(out=ball_t, in_=dr["ball"][:, :])
        bq_t = [ball_t[:, c * 5 + 0:c * 5 + 1] for c in range(CCH)]
        bk_t = [ball_t[:, c * 5 + 1:c * 5 + 2] for c in range(CCH)]
        bo_t = [ball_t[:, c * 5 + 2:c * 5 + 3] for c in range(CCH)]
        A_t = [ball_t[:, c * 5 + 3:c * 5 + 4] for c in range(CCH)]
        B_t = [ball_t[:, c * 5 + 4:c * 5 + 5] for c in range(CCH)]

        onesf = pc.tile([128, 128], F32, tag="onesf", name="onesf")
        nc.vector.memset(onesf, 1.0)
        ones_r = pc.tile([128, 128], F32R, tag="onesr", name="onesr")
        nc.vector.tensor_copy(ones_r[:, :], onesf[:, :])
        e1f = pc.tile([128, 2], F32, tag="e1f", name="e1f")
        nc.vector.memset(e1f, 0.0)
        nc.vector.memset(e1f[0:1, 0:2], 1.0)
        e1_r = pc.tile([128, 2], F32R, tag="e1r", name="e1r")
        nc.vector.tensor_copy(e1_r[:, :], e1f[:, :])
        # warm the Exp table set while the first DMAs stream in
        warmt = pc.tile([128, 1], F32, tag="warmt", name="warmt")
        nc.scalar.activation(warmt[:, :], onesf[:, 0:1], AF.Exp)

        wk_t = [pw.tile([128, C], F32R, tag=f"wk{c}", name=f"wk{c}") for c in range(CCH)]
        wv_t = [pw.tile([128, C], F32R, tag=f"wv{c}", name=f"wv{c}") for c in range(CCH)]
        wo_t = [pw.tile([128, C], F32R, tag=f"wo{c}", name=f"wo{c}") for c in range(CCH)]

        consts = dict(wk_t=wk_t, wv_t=wv_t, wo_t=wo_t,
                      bq_t=bq_t, bk_t=bk_t, bo_t=bo_t, A_t=A_t, B_t=B_t,
                      ones_r=ones_r, e1_r=e1_r, w_loaded=False)
        for _ in range(reps):
            _attn_once(nc, tc, pw, pc, pq, pio, pmm, patt, pdr, dr, consts)
            consts["w_loaded"] = True


def _attn_once(nc, tc, pw, pc, pq, pio, pmm, patt, pdr, dr, cst):
    xf, xs, y = dr["xf"], dr["xs"], dr["y"]
    wk_t, wv_t, wo_t = cst["wk_t"], cst["wv_t"], cst["wo_t"]
    bq_t, bk_t, bo_t = cst["bq_t"], cst["bk_t"], cst["bo_t"]
    A_t, B_t, ones_r = cst["A_t"], cst["B_t"], cst["ones_r"]
    e1_r = cst["e1_r"]

    # DMA queue order at start: first x block, then wv (vT matmuls run first),
    # then wk
    xb_pre = pio.tile([128, CCH, 512], F32, tag="xb", name="xbpre", bufs=2)
    for ci in range(CCH):
        cs = slice(ci * 128, (ci + 1) * 128)
        nc.sync.dma_start(out=xb_pre[:, ci, :], in_=dr["xf"][cs, 0:512])
        if not cst["w_loaded"]:
            nc.sync.dma_start(out=wv_t[ci], in_=dr["wvT"][cs, :].bitcast(F32R))
    if not cst["w_loaded"]:
        for c in range(CCH):
            cs = slice(c * 128, (c + 1) * 128)
            nc.sync.dma_start(out=wk_t[c], in_=dr["wkT"][cs, :].bitcast(F32R))

    with tc.tile_pool(name="pkv", bufs=1) as pkv, \
         tc.tile_pool(name="pacc", bufs=1) as pacc:
        q_t = [pq.tile([128, SL], F32R, tag=f"q{c}", name=f"q{c}")
               for c in range(CCH)]
        acc_t = [[pacc.tile([128, 512], F32R, tag=f"acc{ib}_{co}",
                            name=f"acc{ib}_{co}") for co in range(CCH)]
                 for ib in range(IB)]
        den_t = [pacc.tile([128, 512], F32R, tag=f"den{ib}", name=f"den{ib}")
                 for ib in range(IB)]
        k_t = [pkv.tile([128, JH], F32R, tag=f"k{c}", name=f"k{c}")
               for c in range(CCH)]
        vt_t = [pkv.tile([128, 512], F32R, tag=f"vt{j}", name=f"vt{j}")
                for j in range(JC)]

        def phase_a2_q():
            with tc.tile_pool(name="phns", bufs=1) as phns:
                wq_t = [phns.tile([128, C], F32R, tag=f"wq{c}", name=f"wq{c}")
                        for c in range(CCH)]
                for c in range(CCH):
                    cs = slice(c * 128, (c + 1) * 128)
                    nc.sync.dma_start(out=wq_t[c],
                                      in_=dr["wqT"][cs, :].bitcast(F32R))
                hns = [phns.tile([128, SL], F32R, tag=f"hns{c}", name=f"hns{c}")
                       for c in range(CCH)]
                for c in range(CCH):
                    cs = slice(c * 128, (c + 1) * 128)
                    xst = pio.tile([128, SL], F32, tag="xs", name="xs", bufs=2)
                    nc.sync.dma_start(out=xst, in_=xs[cs, :])
                    nc.vector.tensor_scalar(
                        out=hns[c][:, :], in0=xst[:, :],
                        scalar1=A_t[c], scalar2=B_t[c], op0=OP.mult, op1=OP.add)
                for ib in range(IB):
                    isl = slice(ib * 512, (ib + 1) * 512)
                    for co in range(CCH):
                        qp = pmm.tile([128, 512], F32, tag="mm", name="mm")
                        for ci in range(CCH):
                            nc.tensor.matmul(
                                qp[:, :], wq_t[ci][:, co * 128:(co + 1) * 128],
                                hns[ci][:, isl], start=(ci == 0),
                                stop=(ci == CCH - 1))
                        nc.vector.tensor_scalar(
                            out=q_t[co][:, isl], in0=qp[:, :],
                            scalar1=bq_t[co], scalar2=None, op0=OP.add)

        def kv_production(h):
            for jb in range(JB):
                if h == 0 and jb == 0:
                    xb = xb_pre
                else:
                    j0 = h * JH + jb * 512
                    xb = pio.tile([128, CCH, 512], F32, tag="xb", name="xb",
                                  bufs=2)
                    nc.sync.dma_start(
                        out=xb,
                        in_=bass.AP(tensor=dr["xf"], offset=j0,
                                    ap=[[HW, 128], [128 * HW, CCH], [1, 512]]))
                hnb = []
                for ci in range(CCH):
                    hb = pio.tile([128, 512], F32R, tag=f"hnb{ci}", name="hnb",
                                  bufs=2)
                    nc.vector.tensor_scalar(
                        out=hb[:, :], in0=xb[:, ci, :],
                        scalar1=A_t[ci], scalar2=B_t[ci], op0=OP.mult, op1=OP.add)
                    hnb.append(hb)
                lsl = slice(jb * 512, (jb + 1) * 512)
                for jt in range(4):
                    vp = pmm.tile([128, 512], F32, tag="mm", name="mm")
                    for ci in range(CCH):
                        nc.tensor.matmul(
                            vp[:, :], hnb[ci][:, jt * 128:(jt + 1) * 128],
                            wv_t[ci][:, :], start=(ci == 0), stop=(ci == CCH - 1))
                    nc.vector.tensor_copy(vt_t[jb * 4 + jt][:, :], vp[:, :])
                for co in range(CCH):
                    kp = pmm.tile([128, 512], F32, tag="mm", name="mm")
                    for ci in range(CCH):
                        nc.tensor.matmul(
                            kp[:, :], wk_t[ci][:, co * 128:(co + 1) * 128],
                            hnb[ci][:, :], start=(ci == 0), stop=(ci == CCH - 1))
                    nc.vector.tensor_scalar(
                        out=k_t[co][:, lsl], in0=kp[:, :],
                        scalar1=bk_t[co], scalar2=None, op0=OP.add)

        def attention(h, ib, mid_emit=None):
            isl = slice(ib * 512, (ib + 1) * 512)
            att_ps = [patt.tile([128, 512], F32, tag=f"att{co}",
                                name=f"att{co}") for co in range(CCH)]
            den_ps = patt.tile([128, 512], F32, tag="den", name="den")

            PIPE = 2  # scores/exp groups emitted ahead of their attnV

            def scores(jc):
                sp = pmm.tile([128, 512], F32, tag="mm", name="mm")
                for ci in range(CCH):
                    nc.tensor.matmul(
                        sp[:, :], k_t[ci][:, jc * 128:(jc + 1) * 128],
                        q_t[ci][:, isl], start=(ci == 0), stop=(ci == CCH - 1))
                eT = pio.tile([128, 512], F32R, tag="eT", name="eT", bufs=4)
                nc.scalar.activation(eT[:, :], sp[:, :], AF.Exp,
                                     bias=0.0, scale=SCALE)
                return eT

            eTs = {jc: scores(jc) for jc in range(PIPE)}
            if mid_emit is not None:
                mid_emit()
            for jc in range(JC):
                if jc + PIPE < JC:
                    eTs[jc + PIPE] = scores(jc + PIPE)
                eT = eTs.pop(jc)
                for co in range(CCH):
                    nc.tensor.matmul(
                        att_ps[co][:, :], vt_t[jc][:, co * 128:(co + 1) * 128],
                        eT[:, :], start=(jc == 0), stop=(jc == JC - 1))
                nc.tensor.matmul(
                    den_ps[:, :], ones_r[:, :], eT[:, :],
                    start=(jc == 0), stop=(jc == JC - 1))
            recT = None
            if h == 0:
                nc.scalar.activation(den_t[ib][:, :], den_ps[:, :], AF.Copy,
                                     bias=0.0, scale=1.0)
            else:
                # den first: the reciprocal chain clears the DVE queue before
                # the accumulator adds, so the fused stores never wait on it
                nc.vector.tensor_add(den_t[ib][:, :],
                                     den_t[ib][:, :].bitcast(F32),
                                     den_ps[:, :])
                recT = rec_chain(ib)
            for co in range(CCH):
                if h == 0:
                    if co < 2:
                        nc.scalar.activation(acc_t[ib][co][:, :],
                                             att_ps[co][:, :], AF.Copy,
                                             bias=0.0, scale=1.0)
                    else:
                        nc.vector.tensor_copy(acc_t[ib][co][:, :],
                                              att_ps[co][:, :])
                else:
                    nc.vector.tensor_add(acc_t[ib][co][:, :],
                                         acc_t[ib][co][:, :].bitcast(F32),
                                         att_ps[co][:, :])
            return recT

        def rec_chain(ib):
            # transpose den onto i-partitions: out[i,0] = den[0, it*128+i] via
            # K=1 matmul with the unit vector, then one tiny approx reciprocal
            dT = patt.tile([128, 4, 2], F32, tag="den", name="dT")
            for it in range(4):
                nc.tensor.matmul(
                    dT[:, it, :],
                    den_t[ib][:, it * 128:(it + 1) * 128],
                    e1_r[:, 0:2], start=True, stop=True,
                    skip_group_check=True)
            recT = pio.tile([128, 4, 2], F32, tag="recT", name="recT", bufs=2)
            nc.vector.reciprocal_approx_fast(out=recT[:, :, :], in_=dT[:, :, :])
            return recT

        def finalize(ib, recT):
            # proj in [i, c] layout: lhsT = acc i-slice, rhs = woT chunk;
            # fin = (pp * recT) + (x_slice^T + bo2)  in one fused DVE op
            for it in range(4):
                rows = slice(ib * 512 + it * 128, ib * 512 + (it + 1) * 128)
                pp = pmm.tile([128, 512], F32, tag="mm", name="mm")
                for idx in range(CCH):
                    ci = (it + idx) % CCH
                    nc.tensor.matmul(
                        pp[:, :],
                        acc_t[ib][ci][:, it * 128:(it + 1) * 128],
                        wo_t[ci][:, :], start=(idx == 0), stop=(idx == CCH - 1))
                xrT = pio.tile([128, 512], F32, tag="xr", name="xr", bufs=3)
                nc.sync.dma_start(out=xrT, in_=dr["xsTb"][rows, :])
                fin = pio.tile([128, 512], F32, tag="fin", name="fin", bufs=2)
                nc.vector.scalar_tensor_tensor(
                    out=fin[:, :], in0=pp[:, :], scalar=recT[:, it, 0:1],
                    in1=xrT[:, :], op0=OP.mult, op1=OP.add)
                nc.sync.dma_start(out=y[rows, :], in_=fin[:, :])

        kv_production(0)
        phase_a2_q()
        if not cst["w_loaded"]:
            for c in range(CCH):
                cs = slice(c * 128, (c + 1) * 128)
                nc.sync.dma_start(out=wo_t[c],
                                  in_=dr["woT"][cs, :].bitcast(F32R))
        attention(0, 0)
        attention(0, 1)
        kv_production(1)
        rb0 = attention(1, 0)
        rb1 = attention(1, 1, mid_emit=lambda: finalize(0, rb0))
        finalize(1, rb1)


_NC_CACHE = {}


def _get_nc(reps: int = 1):
    if reps not in _NC_CACHE:
        _NC_CACHE[reps] = build(reps)
    return _NC_CACHE[reps]


def _host_inputs(x, norm_gamma, norm_beta, wq, bq, wk, bk, wv, bv, wo, bo):
    f32, f64 = np.float32, np.float64
    wqT = np.ascontiguousarray(np.asarray(wq, f32).T)
    wkT = np.ascontiguousarray(np.asarray(wk, f32).T)
    wvT = np.ascontiguousarray(np.asarray(wv, f32).T)
    woT = np.ascontiguousarray(np.asarray(wo, f32).T)
    bo2 = np.asarray(bo, f64) + np.asarray(wo, f64) @ np.asarray(bv, f64)

    x = np.asarray(x, f32)
    gamma = np.asarray(norm_gamma, f64)
    beta = np.asarray(norm_beta, f64)
    shared = {"wqT": wqT, "wkT": wkT, "wvT": wvT, "woT": woT}
    in_maps = []
    for core in range(NCORES):
        b, s = core // NSLICE, core % NSLICE
        xfb = np.ascontiguousarray(x[b].reshape(C, HW))
        xsb = np.ascontiguousarray(xfb[:, s * SL:(s + 1) * SL])
        # GroupNorm affine per channel for this batch (fp64 host stats)
        xg = xfb.astype(f64).reshape(NG, (C // NG) * HW)
        mean = xg.mean(axis=1)
        var = xg.var(axis=1)
        rstd = 1.0 / np.sqrt(var + EPS)
        gmat = gamma.reshape(NG, C // NG)
        Ag = (gmat * rstd[:, None]).reshape(C)
        Bg = (beta.reshape(NG, C // NG)
              - mean[:, None] * gmat * rstd[:, None]).reshape(C)
        ball = np.stack([np.asarray(bq, f64), np.asarray(bk, f64), bo2,
                         Ag, Bg], axis=1)
        ball = ball.reshape(CCH, 128, 5).transpose(1, 0, 2).reshape(128, CCH * 5)
        xsTb = np.ascontiguousarray(xsb.T.astype(f64) + bo2[None, :], f32)
        in_maps.append(dict(shared, xf=xfb, xs=xsb, xsTb=xsTb,
                            ball=np.ascontiguousarray(ball, f32)))
    return in_maps


def kernel(x, norm_gamma, norm_beta, wq, bq, wk, bk, wv, bv, wo, bo,
           reps: int = 1):
    nc = _get_nc(reps)
    in_maps = _host_inputs(x, norm_gamma, norm_beta, wq, bq, wk, bk, wv, bv,
                           wo, bo)
    res = run_bass_kernel_spmd(nc, in_maps, core_ids=list(range(NCORES)),
                               trace=False)
    out = np.empty((B, C, HW), np.float32)
    for core in range(NCORES):
        b, s = core // NSLICE, core % NSLICE
        out[b][:, s * SL:(s + 1) * SL] = res.results[core]["y"].T
    return out.reshape(B, C, H, W)



# revision 31
# speedup vs baseline: 2.4090x; 2.4090x over previous
"""AttnBlock (GroupNorm -> QKV -> 4096x4096 spatial attention -> proj -> residual)
for Trainium2, sharded over 8 NeuronCores. fp8e4m3 DoubleRow edition.

Sharding: core = (batch b, query-slice s); b = core//4, s = core%4. Each core
computes K/V for its full batch image (redundant across the 4 cores of a
batch) and attention/projection for its 1024-query slice. No collectives.

Host-side prep (exact, in f64/f32):
  - GroupNorm is folded into the weights: A = gamma*rstd, B = beta - mean*A
    (per batch); wq' = 8*(wq . diag(A)) etc., so the device consumes raw x.
  - x is quantized to fp8 on host (2MB/core instead of 8MB), and its columns
    are rotated by the core's query-slice offset so the SPMD program always
    reads its queries from local columns [0, 1024) (softmax is j-order
    invariant, so K/V order doesn't matter).
  - The K bias drops entirely: softmax(q.(k+bk)) == softmax(q.k + const_j).
  - All weights are pre-scaled by 8 to center fp8 quantization; the exp is
    shifted by -2 (softmax-invariant) to keep e^s inside fp8 range; both
    factors cancel exactly through the final reciprocal/projection scaling.

Device math (all matmuls fp8e4m3 with MatmulPerfMode.DoubleRow: K=256 per
instruction at 0.5 cycles/row = 4x the f32r rate):
  k = wk'@x8, v = wv'@x8, q = wq'@x8 + tq  (KV psums packed in pairs inside
  [128,4,256] quad banks -> one 1024-col fp8 cast per pair on ACT/DVE)
  eT[j,i] = fp8(exp(k^T q * scale - 2))    (one ACT exp per 4-chunk quad)
  att[c,i] += v eT ; den[i] += 1^T eT      (PSUM accumulation over all j)
  out = (wo'@(att/512)) * (8/den) + (x^T + bo2)   (proj + fused residual)
Queries processed in four 256-wide i-blocks; the first is fused into KV
production, the rest stream afterwards. K/V/x resident in SBUF as fp8.
"""
import numpy as np
import ml_dtypes
import concourse.bacc as bacc
import concourse.bass as bass
import concourse.tile as tile
import concourse.mybir as mybir
from concourse.bass_utils import run_bass_kernel_spmd

F32 = mybir.dt.float32
F32R = mybir.dt.float32r
FP8 = mybir.dt.float8e4
AF = mybir.ActivationFunctionType
OP = mybir.AluOpType
DR = mybir.MatmulPerfMode.DoubleRow
E4 = ml_dtypes.float8_e4m3

B, C, H, W = 2, 512, 64, 64
HW = H * W                    # 4096
NCORES = 8
NSLICE = 4                    # query slices per batch
SL = HW // NSLICE             # 1024 query positions per core
NG = 32                       # groups
EPS = 1e-6
CCH = C // 128                # 4 channel chunks
NT = CCH // 2                 # 2 chunk-pairs per C contraction (DoubleRow)
JB = HW // 512                # 8 j-blocks
JC = HW // 128                # 32 j-chunks
JP = JC // 2                  # 16 j-pairs of 256
NCALL = 4                     # i-blocks of 256 per core
IBW = SL // NCALL             # 256
WS = 8.0                      # host weight prescale
SHIFT = -2.0                  # exp bias (softmax-invariant)
S_AO = 1.0 / 512              # att -> fp8 cast scale
E1V = 0.125                   # rec transpose scale -> rec = 8/den
SC2 = float(C) ** -0.5 / (WS * WS)
DBG_STUB_REC = False
# engine maps ("a"=ACT, "v"=DVE); Pool/gpsimd cannot read PSUM on trn2.
# KV_ENG: engine of the four wide kv casts per jb (qk1, qk2, qv1, qv2)
KV_ENG = {}
KV_ENG_LATE = "vava"
AO_ENG = {0: "vv", 1: "vv", 2: "vv", 3: "av"}
ATT_LAG = 3
FUSE0 = True
KV_SPLIT = False  # split each wide kv cast across both engines


def build(reps: int = 1):
    nc = bacc.Bacc("TRN2", target_bir_lowering=False)
    dr = {}
    dr["x8"] = nc.dram_tensor("x8", [128, CCH, HW], FP8, kind="ExternalInput")
    for w in ("wq8", "wk8", "wv8", "wo8"):
        dr[w] = nc.dram_tensor(w, [128, CCH, C], FP8, kind="ExternalInput")
    dr["tqb"] = nc.dram_tensor("tqb", [128, CCH], F32, kind="ExternalInput")
    dr["xrT"] = nc.dram_tensor("xrT", [SL, C], F32, kind="ExternalInput")
    dr["y"] = nc.dram_tensor("y", [SL, C], F32, kind="ExternalOutput")
    with tile.TileContext(nc) as tc:
        _body(nc, tc, reps, dr)
    nc.finalize()
    return nc


def _body(nc, tc, reps, dr):
    from contextlib import ExitStack
    with ExitStack() as ctx:
        pc = ctx.enter_context(tc.tile_pool(name="pc", bufs=1))
        pio = ctx.enter_context(tc.tile_pool(name="pio", bufs=1))
        pquad = ctx.enter_context(tc.tile_pool(name="pquad", bufs=2,
                                               space="PSUM"))
        pmm = ctx.enter_context(tc.tile_pool(name="pmm", bufs=1, space="PSUM"))
        patt = ctx.enter_context(tc.tile_pool(name="patt", bufs=1,
                                              space="PSUM"))

        # small consts
        ones8 = pc.tile([128, 2, 128], FP8, tag="ones8", name="ones8")
        nc.vector.memset(ones8, 1.0)
        # dummy matmuls: anchor the PE p-state ramp during the DMA fill so
        # real matmuls run at full clock from the start
        wps = pquad.tile([128, 4, 256], F32, tag="qd", name="warmmm")
        for i in range(64):
            nc.tensor.matmul(wps[:, 0, 0:128], ones8[:, :, :], ones8[:, :, :],
                             start=(i == 0), stop=(i == 63), perf_mode=DR)
        e1f = pc.tile([128, 2], F32, tag="e1f", name="e1f")
        nc.vector.memset(e1f, 0.0)
        nc.vector.memset(e1f[0:1, 0:2], E1V)
        e1r = pc.tile([128, 2], F32R, tag="e1r", name="e1r")
        nc.vector.tensor_copy(e1r[:, :], e1f[:, :])
        bsh = pc.tile([128, 1], F32, tag="bsh", name="bsh")
        nc.vector.memset(bsh, SHIFT)
        warm = pc.tile([128, 1], F32, tag="warm", name="warm")
        nc.scalar.activation(warm[:, :], bsh[:, 0:1], AF.Exp)

        tq = pc.tile([128, CCH], F32, tag="tq", name="tq")

        # persistent fp8 operands
        x8 = pc.tile([128, CCH, HW], FP8, tag="x8", name="x8")
        k8 = pc.tile([128, CCH, HW], FP8, tag="k8", name="k8")
        q8 = pc.tile([128, CCH, SL], FP8, tag="q8", name="q8")
        w8 = {}
        for w in ("wq8", "wk8", "wv8", "wo8"):
            w8[w] = pc.tile([128, CCH, C], FP8, tag=w, name=w)
        vt = [pc.tile([128, 2, 512], FP8, tag=f"vt{j}", name=f"vt{j}")
              for j in range(JP)]
        xr = [pc.tile([128, C], F32, tag=f"xr{i}", name=f"xr{i}")
              for i in range(8)]

        # input DMAs in consumption order
        def dma_x8_cols(c0, c1):
            nc.sync.dma_start(
                out=x8[:, :, c0:c1],
                in_=bass.AP(tensor=dr["x8"], offset=c0,
                            ap=[[CCH * HW, 128], [HW, CCH], [1, c1 - c0]]))

        def dma_w(name):
            nc.sync.dma_start(
                out=w8[name],
                in_=bass.AP(tensor=dr[name], offset=0,
                            ap=[[CCH * C, 128], [C, CCH], [1, C]]))

        dma_x8_cols(0, 512)
        dma_w("wk8")
        dma_w("wv8")
        dma_x8_cols(512, 1024)
        dma_w("wq8")
        nc.sync.dma_start(out=tq, in_=dr["tqb"][:, :])
        for c in range(1, 4):
            dma_x8_cols(c * 1024, (c + 1) * 1024)
        dma_w("wo8")
        for i in range(8):
            nc.sync.dma_start(
                out=xr[i], in_=dr["xrT"][i * 128:(i + 1) * 128, :])

        # PSUM accumulation banks: att chunks packed two per bank
        attp = [patt.tile([128, 2, IBW], F32, tag=f"att{t}", name=f"att{t}")
                for t in range(NT)]
        den_ps = patt.tile([128, IBW], F32, tag="den", name="den")

        cst = dict(ones8=ones8, e1r=e1r, bsh=bsh, tq=tq, x8=x8, k8=k8, q8=q8,
                   w8=w8, vt=vt, xr=xr, attp=attp, den_ps=den_ps)
        for _ in range(reps):
            _attn_once(nc, tc, pc, pio, pquad, pmm, patt, dr, cst)


def _attn_once(nc, tc, pc, pio, pquad, pmm, patt, dr, cst):
    from collections import deque
    x8, k8, q8, w8 = cst["x8"], cst["k8"], cst["q8"], cst["w8"]
    vt, xr, attp, den_ps = cst["vt"], cst["xr"], cst["attp"], cst["den_ps"]
    ones8, e1r, bsh, tq = cst["ones8"], cst["e1r"], cst["bsh"], cst["tq"]

    def cast(eng, out, in_):
        if eng == "a":
            nc.scalar.copy(out, in_)
        else:
            nc.vector.tensor_copy(out, in_)

    def kv(jb):
        engs = KV_ENG.get(jb, KV_ENG_LATE)
        jsl = slice(jb * 512, (jb + 1) * 512)
        for h in range(2):  # K co-pairs (0,1) and (2,3)
            qk = pquad.tile([128, 4, 256], F32, tag="qd", name="qd")
            for g in range(2):
                co = 2 * h + g
                for t in range(NT):
                    nc.tensor.matmul(
                        qk[:, 2 * g:2 * g + 2, :],
                        w8["wk8"][:, 2 * t:2 * t + 2, co * 128:(co + 1) * 128],
                        x8[:, 2 * t:2 * t + 2, jsl], start=(t == 0),
                        stop=(t == NT - 1), perf_mode=DR)
            if KV_SPLIT:
                e0 = engs[h]
                e1 = "a" if e0 == "v" else "v"
                cast(e0, k8[:, 2 * h, jsl], qk[:, 0:2, :])
                cast(e1, k8[:, 2 * h + 1, jsl], qk[:, 2:4, :])
            else:
                cast(engs[h], k8[:, 2 * h:2 * h + 2, jsl], qk[:, :, :])
        for h in range(2):  # V jt-pairs -> vt tiles
            qv = pquad.tile([128, 4, 256], F32, tag="qd", name="qd")
            for g in range(2):
                jt = 2 * h + g
                j0 = jb * 512 + jt * 128
                for t in range(NT):
                    nc.tensor.matmul(
                        qv[:, 2 * g:2 * g + 2, :],
                        x8[:, 2 * t:2 * t + 2, j0:j0 + 128],
                        w8["wv8"][:, 2 * t:2 * t + 2, :], start=(t == 0),
                        stop=(t == NT - 1), perf_mode=DR)
            if KV_SPLIT:
                e0 = engs[2 + h]
                e1 = "a" if e0 == "v" else "v"
                cast(e0, vt[2 * jb + h][:, 0, :], qv[:, 0:2, :])
                cast(e1, vt[2 * jb + h][:, 1, :], qv[:, 2:4, :])
            else:
                cast(engs[2 + h], vt[2 * jb + h][:, :, :], qv[:, :, :])

    def qprod(ib):
        isl = slice(ib * 512, (ib + 1) * 512)
        for h in range(2):
            qq = pquad.tile([128, 4, 256], F32, tag="qd", name="qd")
            for g in range(2):
                co = 2 * h + g
                for t in range(NT):
                    nc.tensor.matmul(
                        qq[:, 2 * g:2 * g + 2, :],
                        w8["wq8"][:, 2 * t:2 * t + 2, co * 128:(co + 1) * 128],
                        x8[:, 2 * t:2 * t + 2, isl], start=(t == 0),
                        stop=(t == NT - 1), perf_mode=DR)
                nc.scalar.activation(q8[:, co, isl], qq[:, 2 * g:2 * g + 2, :],
                                     AF.Identity, bias=tq[:, co:co + 1],
                                     scale=1.0)

    def scores_quad(ci, qi):
        isl = slice(ci * IBW, (ci + 1) * IBW)
        sq = pquad.tile([128, 4, 256], F32, tag="qd", name="qd")
        for qj in range(4):
            jc = 4 * qi + qj
            for t in range(NT):
                nc.tensor.matmul(
                    sq[:, qj, :], k8[:, 2 * t:2 * t + 2, jc * 128:(jc + 1) * 128],
                    q8[:, 2 * t:2 * t + 2, isl], start=(t == 0),
                    stop=(t == NT - 1), perf_mode=DR)
        eT = pio.tile([128, 4, 256], FP8, tag="eT", name="eT", bufs=4)
        nc.scalar.activation(eT[:, :, :], sq[:, :, :], AF.Exp,
                             bias=bsh[:, 0:1], scale=SC2)
        return eT

    def attnv_quad(qi, eT, st):
        for pr in range(2):
            jp = 2 * qi + pr
            first = st["n"] == 0
            last = st["n"] == JP - 1
            st["n"] += 1
            for co in range(CCH):
                nc.tensor.matmul(
                    attp[co // 2][:, co % 2, :],
                    vt[jp][:, :, co * 128:(co + 1) * 128],
                    eT[:, 2 * pr:2 * pr + 2, :], start=first, stop=last,
                    perf_mode=DR)
            nc.tensor.matmul(den_ps[:, :], ones8[:, :, :],
                             eT[:, 2 * pr:2 * pr + 2, :], start=first,
                             stop=last, perf_mode=DR)

    def att_feed(ci, qis, st, depth=1):
        for qi in qis:
            st["q"].append((qi, scores_quad(ci, qi)))
            while len(st["q"]) > depth:
                qi0, eT0 = st["q"].popleft()
                attnv_quad(qi0, eT0, st)

    def att_flush(st):
        while st["q"]:
            qi0, eT0 = st["q"].popleft()
            attnv_quad(qi0, eT0, st)

    def den_rec():
        dsb = pio.tile([128, IBW], F32R, tag="dsb", name="dsb", bufs=2)
        nc.vector.tensor_copy(dsb[:, :], den_ps[:, :])
        rec = pio.tile([128, 4], F32, tag="rec", name="rec", bufs=2)
        if DBG_STUB_REC:
            nc.vector.memset(rec, 0.001)
            return rec
        dT = pmm.tile([128, 512], F32, tag="mm", name="dT")
        for it in range(2):
            nc.tensor.matmul(dT[:, 2 * it:2 * it + 2],
                             dsb[:, it * 128:(it + 1) * 128],
                             e1r[:, 0:2], start=True, stop=True,
                             skip_group_check=True)
        nc.vector.reciprocal_approx_fast(out=rec[:, 0:4], in_=dT[:, 0:4])
        return rec

    def ao_cast(ci):
        ao = [pio.tile([128, 2, IBW], FP8, tag=f"ao{t}", name=f"ao{t}",
                       bufs=4) for t in range(NT)]
        for t in range(NT):
            if AO_ENG[ci][t] == "a":
                nc.scalar.activation(ao[t][:, :, :], attp[t][:, :, :],
                                     AF.Copy, bias=0.0, scale=S_AO)
            else:
                nc.vector.tensor_scalar(out=ao[t][:, :, :],
                                        in0=attp[t][:, :, :],
                                        scalar1=S_AO, scalar2=None,
                                        op0=OP.mult)
        return ao

    def finalize(ci, rec, ao):
        for it in range(2):
            rows = slice(ci * IBW + it * 128, ci * IBW + (it + 1) * 128)
            pp = pmm.tile([128, 512], F32, tag="mm", name="mm")
            for t in range(NT):
                nc.tensor.matmul(
                    pp[:, :], ao[t][:, :, it * 128:(it + 1) * 128],
                    w8["wo8"][:, 2 * t:2 * t + 2, :], start=(t == 0),
                    stop=(t == NT - 1), perf_mode=DR)
            fin = pio.tile([128, 512], F32, tag="fin", name="fin", bufs=3)
            nc.vector.scalar_tensor_tensor(
                out=fin[:, :], in0=pp[:, :], scalar=rec[:, 2 * it:2 * it + 1],
                in1=xr[ci * 2 + it][:, :], op0=OP.mult, op1=OP.add)
            nc.sync.dma_start(out=dr["y"][rows, :], in_=fin[:, :])

    # ---- emission schedule ----
    # call 0 (i 0:256) is fused into KV production, lagging ATT_LAG blocks so
    # the wide kv casts drain while PE runs attention on older blocks
    st0 = {"q": deque(), "n": 0}
    if FUSE0:
        kv(0)
        qprod(0)
        kv(1)
        qprod(1)
        kv(2)
        for jb in range(JB - ATT_LAG):
            att_feed(0, [jb], st0)
            kv(jb + ATT_LAG)
        for jb in range(JB - ATT_LAG, JB):
            att_feed(0, [jb], st0)
    else:
        kv(0)
        qprod(0)
        kv(1)
        qprod(1)
        for jb in range(2, JB):
            kv(jb)
        att_feed(0, range(0, JB), st0)
    att_flush(st0)
    recs = {0: den_rec()}
    aos = {0: ao_cast(0)}
    for ci in range(1, NCALL):
        st = {"q": deque(), "n": 0}
        att_feed(ci, range(0, 3), st)
        finalize(ci - 1, recs[ci - 1], aos[ci - 1])
        att_feed(ci, range(3, JB), st)
        att_flush(st)
        recs[ci] = den_rec()
        aos[ci] = ao_cast(ci)
    finalize(NCALL - 1, recs[NCALL - 1], aos[NCALL - 1])


_NC_CACHE = {}


def _get_nc(reps: int = 1):
    if reps not in _NC_CACHE:
        _NC_CACHE[reps] = build(reps)
    return _NC_CACHE[reps]


def _pack_w(wt):
    """[Cout, Cin] (already scaled) -> [128, CCH, Cout] fp8, [p, ci, co] =
    wt[co, ci*128+p]."""
    return np.ascontiguousarray(
        wt.T.reshape(CCH, 128, C).transpose(1, 0, 2)).astype(E4)


def _host_inputs(x, norm_gamma, norm_beta, wq, bq, wk, bk, wv, bv, wo, bo):
    f32, f64 = np.float32, np.float64
    x = np.asarray(x, f32)
    gamma = np.asarray(norm_gamma, f64)
    beta = np.asarray(norm_beta, f64)
    wqd, bqd = np.asarray(wq, f64), np.asarray(bq, f64)
    wkd = np.asarray(wk, f64)
    wvd, bvd = np.asarray(wv, f64), np.asarray(bv, f64)
    wod, bod = np.asarray(wo, f64), np.asarray(bo, f64)

    in_maps = []
    per_batch = {}
    for b in range(B):
        xb = x[b].reshape(C, HW)
        xg = xb.astype(f64).reshape(NG, (C // NG) * HW)
        mean, var = xg.mean(axis=1), xg.var(axis=1)
        rstd = 1.0 / np.sqrt(var + EPS)
        gmat = gamma.reshape(NG, C // NG)
        A = (gmat * rstd[:, None]).reshape(C)
        Bv = (beta.reshape(NG, C // NG) - (mean * rstd)[:, None] * gmat
              ).reshape(C)
        tqv = WS * (bqd + wqd @ Bv)
        bo2 = bod + wod @ (bvd + wvd @ Bv)
        per_batch[b] = dict(
            xb=xb,
            wq8=_pack_w(WS * wqd * A[None, :]),
            wk8=_pack_w(WS * wkd * A[None, :]),
            wv8=_pack_w(WS * wvd * A[None, :]),
            wo8=_pack_w(WS * wod),
            tqb=np.ascontiguousarray(tqv.reshape(CCH, 128).T, dtype=f32),
            bo2=bo2,
        )
    for core in range(NCORES):
        b, s = core // NSLICE, core % NSLICE
        pb = per_batch[b]
        xb = pb["xb"]
        xl = np.roll(xb, -s * SL, axis=1)  # local j order: queries at cols 0..SL
        x8 = np.ascontiguousarray(
            xl.reshape(CCH, 128, HW).transpose(1, 0, 2)).astype(E4)
        xrT = np.ascontiguousarray(
            xb[:, s * SL:(s + 1) * SL].T.astype(f64) + pb["bo2"][None, :], f32)
        in_maps.append(dict(x8=x8, wq8=pb["wq8"], wk8=pb["wk8"],
                            wv8=pb["wv8"], wo8=pb["wo8"], tqb=pb["tqb"],
                            xrT=xrT))
    return in_maps


def kernel(x, norm_gamma, norm_beta, wq, bq, wk, bk, wv, bv, wo, bo,
           reps: int = 1):
    nc = _get_nc(reps)
    in_maps = _host_inputs(x, norm_gamma, norm_beta, wq, bq, wk, bk, wv, bv,
                           wo, bo)
    res = run_bass_kernel_spmd(nc, in_maps, core_ids=list(range(NCORES)),
                               trace=False)
    out = np.empty((B, C, HW), np.float32)
    for core in range(NCORES):
        b, s = core // NSLICE, core % NSLICE
        out[b][:, s * SL:(s + 1) * SL] = res.results[core]["y"].T
    return out.reshape(B, C, H, W)


# revision 37
# speedup vs baseline: 2.7388x; 1.1369x over previous
"""AttnBlock (GroupNorm -> QKV -> 4096x4096 spatial attention -> proj -> residual)
for Trainium2, sharded over 8 NeuronCores. fp8e4m3 DoubleRow edition.

Sharding: core = (batch b, query-slice s); b = core//4, s = core%4. Each core
computes K/V for its full batch image (redundant across the 4 cores of a
batch) and attention/projection for its 1024-query slice. No collectives.

Host-side prep (exact, in f64/f32):
  - GroupNorm is folded into the weights: A = gamma*rstd, B = beta - mean*A
    (per batch); wq' = 8*(wq . diag(A)) etc., so the device consumes raw x.
  - x is quantized to fp8 on host (2MB/core instead of 8MB), and its columns
    are rotated by the core's query-slice offset so the SPMD program always
    reads its queries from local columns [0, 1024) (softmax is j-order
    invariant, so K/V order doesn't matter).
  - The K bias drops entirely: softmax(q.(k+bk)) == softmax(q.k + const_j).
  - All weights are pre-scaled by 8 to center fp8 quantization; the exp is
    shifted by -2 (softmax-invariant) to keep e^s inside fp8 range; both
    factors cancel exactly through the final reciprocal/projection scaling.

Device math (all matmuls fp8e4m3 with MatmulPerfMode.DoubleRow: K=256 per
instruction at 0.5 cycles/row = 4x the f32r rate):
  k = wk'@x8, v = wv'@x8, q = wq'@x8 + tq  (KV psums packed in pairs inside
  [128,4,256] quad banks -> one 1024-col fp8 cast per pair on ACT/DVE)
  eT[j,i] = fp8(exp(k^T q * scale - 2))    (one ACT exp per 4-chunk quad)
  att[c,i] += v eT ; den[i] += 1^T eT      (PSUM accumulation over all j)
  out = (wo'@(att/512)) * (8/den) + (x^T + bo2)   (proj + fused residual)
Queries processed in four 256-wide i-blocks; the first is fused into KV
production, the rest stream afterwards. K/V/x resident in SBUF as fp8.
"""
import numpy as np
import ml_dtypes
import concourse.bacc as bacc
import concourse.bass as bass
import concourse.tile as tile
import concourse.mybir as mybir
from concourse.bass_utils import run_bass_kernel_spmd

F32 = mybir.dt.float32
F32R = mybir.dt.float32r
FP8 = mybir.dt.float8e4
AF = mybir.ActivationFunctionType
OP = mybir.AluOpType
DR = mybir.MatmulPerfMode.DoubleRow
E4 = ml_dtypes.float8_e4m3

B, C, H, W = 2, 512, 64, 64
HW = H * W                    # 4096
NCORES = 8
NSLICE = 4                    # query slices per batch
SL = HW // NSLICE             # 1024 query positions per core
NG = 32                       # groups
EPS = 1e-6
CCH = C // 128                # 4 channel chunks
NT = CCH // 2                 # 2 chunk-pairs per C contraction (DoubleRow)
JB = HW // 512                # 8 j-blocks
JC = HW // 128                # 32 j-chunks
JP = JC // 2                  # 16 j-pairs of 256
NCALL = 4                     # i-blocks of 256 per core
IBW = SL // NCALL             # 256
WS = 8.0                      # host weight prescale
SHIFT = -2.0                  # exp bias (softmax-invariant)
S_AO = 1.0 / 512              # att -> fp8 cast scale
E1V = 0.125                   # rec transpose scale -> rec = 8/den
SC2 = float(C) ** -0.5 / (WS * WS)
DBG_STUB_REC = False
# engine maps ("a"=ACT, "v"=DVE); Pool/gpsimd cannot read PSUM on trn2.
# KV_ENG: engine of the four wide kv casts per jb (qk1, qk2, qv1, qv2)
KV_ENG = {}
KV_ENG_LATE = "vava"
AO_ENG = {0: "vv", 1: "vv", 2: "vv", 3: "av"}
ATT_LAG = 3
FUSE0 = True
KV_SPLIT = False  # split each wide kv cast across both engines
KV_BUFS = 4       # scope-A quad rotation depth


def build(reps: int = 1):
    nc = bacc.Bacc("TRN2", target_bir_lowering=False)
    dr = {}
    dr["x8"] = nc.dram_tensor("x8", [128, CCH, HW], FP8, kind="ExternalInput")
    for w in ("wq8", "wk8", "wv8", "wo8"):
        dr[w] = nc.dram_tensor(w, [128, CCH, C], FP8, kind="ExternalInput")
    dr["tqb"] = nc.dram_tensor("tqb", [128, CCH], F32, kind="ExternalInput")
    dr["xrT"] = nc.dram_tensor("xrT", [SL, C], F32, kind="ExternalInput")
    dr["y"] = nc.dram_tensor("y", [SL, C], F32, kind="ExternalOutput")
    with tile.TileContext(nc) as tc:
        _body(nc, tc, reps, dr)
    nc.finalize()
    return nc


def _body(nc, tc, reps, dr):
    from contextlib import ExitStack
    with ExitStack() as ctx:
        pc = ctx.enter_context(tc.tile_pool(name="pc", bufs=1))
        pio = ctx.enter_context(tc.tile_pool(name="pio", bufs=1))

        # small consts
        ones8 = pc.tile([128, 2, 128], FP8, tag="ones8", name="ones8")
        nc.vector.memset(ones8, 1.0)
        e1f = pc.tile([128, 2], F32, tag="e1f", name="e1f")
        nc.vector.memset(e1f, 0.0)
        nc.vector.memset(e1f[0:1, 0:2], E1V)
        e1r = pc.tile([128, 2], F32R, tag="e1r", name="e1r")
        nc.vector.tensor_copy(e1r[:, :], e1f[:, :])
        bsh = pc.tile([128, 1], F32, tag="bsh", name="bsh")
        nc.vector.memset(bsh, SHIFT)
        warm = pc.tile([128, 1], F32, tag="warm", name="warm")
        nc.scalar.activation(warm[:, :], bsh[:, 0:1], AF.Exp)

        tq = pc.tile([128, CCH], F32, tag="tq", name="tq")

        # persistent fp8 operands
        x8 = pc.tile([128, CCH, HW], FP8, tag="x8", name="x8")
        k8 = pc.tile([128, CCH, HW], FP8, tag="k8", name="k8")
        q8 = pc.tile([128, CCH, SL], FP8, tag="q8", name="q8")
        w8 = {}
        for w in ("wq8", "wk8", "wv8", "wo8"):
            w8[w] = pc.tile([128, CCH, C], FP8, tag=w, name=w)
        vt = [pc.tile([128, 2, 512], FP8, tag=f"vt{j}", name=f"vt{j}")
              for j in range(JP)]
        xr = [pc.tile([128, C], F32, tag=f"xr{i}", name=f"xr{i}")
              for i in range(8)]

        # input DMAs in consumption order
        def dma_x8_cols(c0, c1):
            nc.sync.dma_start(
                out=x8[:, :, c0:c1],
                in_=bass.AP(tensor=dr["x8"], offset=c0,
                            ap=[[CCH * HW, 128], [HW, CCH], [1, c1 - c0]]))

        def dma_w(name):
            nc.sync.dma_start(
                out=w8[name],
                in_=bass.AP(tensor=dr[name], offset=0,
                            ap=[[CCH * C, 128], [C, CCH], [1, C]]))

        dma_x8_cols(0, 512)
        dma_w("wk8")
        dma_w("wv8")
        dma_x8_cols(512, 1024)
        dma_w("wq8")
        nc.sync.dma_start(out=tq, in_=dr["tqb"][:, :])
        for c in range(1, 4):
            dma_x8_cols(c * 1024, (c + 1) * 1024)
        dma_w("wo8")
        for i in range(8):
            nc.sync.dma_start(
                out=xr[i], in_=dr["xrT"][i * 128:(i + 1) * 128, :])

        cst = dict(ones8=ones8, e1r=e1r, bsh=bsh, tq=tq, x8=x8, k8=k8, q8=q8,
                   w8=w8, vt=vt, xr=xr)
        for _ in range(reps):
            _attn_once(nc, tc, pc, pio, dr, cst)


def _attn_once(nc, tc, pc, pio, dr, cst):
    from collections import deque
    x8, k8, q8, w8 = cst["x8"], cst["k8"], cst["q8"], cst["w8"]
    vt, xr = cst["vt"], cst["xr"]
    ones8, e1r, bsh, tq = cst["ones8"], cst["e1r"], cst["bsh"], cst["tq"]
    pools = {}

    def pquad_tile(name="qd"):
        return pools["pq"].tile([128, 4, 256], F32, tag="qd", name=name)

    def cast(eng, out, in_):
        if eng == "a":
            nc.scalar.copy(out, in_)
        else:
            nc.vector.tensor_copy(out, in_)

    def kv(jb):
        engs = KV_ENG.get(jb, KV_ENG_LATE)
        jsl = slice(jb * 512, (jb + 1) * 512)
        for h in range(2):  # K co-pairs (0,1) and (2,3)
            qk = pquad_tile()
            for g in range(2):
                co = 2 * h + g
                for t in range(NT):
                    nc.tensor.matmul(
                        qk[:, 2 * g:2 * g + 2, :],
                        w8["wk8"][:, 2 * t:2 * t + 2, co * 128:(co + 1) * 128],
                        x8[:, 2 * t:2 * t + 2, jsl], start=(t == 0),
                        stop=(t == NT - 1), perf_mode=DR)
            if KV_SPLIT:
                e0 = engs[h]
                e1 = "a" if e0 == "v" else "v"
                cast(e0, k8[:, 2 * h, jsl], qk[:, 0:2, :])
                cast(e1, k8[:, 2 * h + 1, jsl], qk[:, 2:4, :])
            else:
                cast(engs[h], k8[:, 2 * h:2 * h + 2, jsl], qk[:, :, :])
        for h in range(2):  # V jt-pairs -> vt tiles
            qv = pquad_tile()
            for g in range(2):
                jt = 2 * h + g
                j0 = jb * 512 + jt * 128
                for t in range(NT):
                    nc.tensor.matmul(
                        qv[:, 2 * g:2 * g + 2, :],
                        x8[:, 2 * t:2 * t + 2, j0:j0 + 128],
                        w8["wv8"][:, 2 * t:2 * t + 2, :], start=(t == 0),
                        stop=(t == NT - 1), perf_mode=DR)
            if KV_SPLIT:
                e0 = engs[2 + h]
                e1 = "a" if e0 == "v" else "v"
                cast(e0, vt[2 * jb + h][:, 0, :], qv[:, 0:2, :])
                cast(e1, vt[2 * jb + h][:, 1, :], qv[:, 2:4, :])
            else:
                cast(engs[2 + h], vt[2 * jb + h][:, :, :], qv[:, :, :])

    def qprod(ib):
        isl = slice(ib * 512, (ib + 1) * 512)
        for h in range(2):
            qq = pquad_tile()
            for g in range(2):
                co = 2 * h + g
                for t in range(NT):
                    nc.tensor.matmul(
                        qq[:, 2 * g:2 * g + 2, :],
                        w8["wq8"][:, 2 * t:2 * t + 2, co * 128:(co + 1) * 128],
                        x8[:, 2 * t:2 * t + 2, isl], start=(t == 0),
                        stop=(t == NT - 1), perf_mode=DR)
                nc.scalar.activation(q8[:, co, isl], qq[:, 2 * g:2 * g + 2, :],
                                     AF.Identity, bias=tq[:, co:co + 1],
                                     scale=1.0)

    def scores_quad(ci, qi):
        isl = slice(ci * IBW, (ci + 1) * IBW)
        sq = pquad_tile()
        for qj in range(4):
            jc = 4 * qi + qj
            for t in range(NT):
                nc.tensor.matmul(
                    sq[:, qj, :], k8[:, 2 * t:2 * t + 2, jc * 128:(jc + 1) * 128],
                    q8[:, 2 * t:2 * t + 2, isl], start=(t == 0),
                    stop=(t == NT - 1), perf_mode=DR)
        eT = pio.tile([128, 4, 256], FP8, tag="eT", name="eT", bufs=4)
        nc.scalar.activation(eT[:, :, :], sq[:, :, :], AF.Exp,
                             bias=bsh[:, 0:1], scale=SC2)
        return eT

    def attnv_quad(qi, eT, st):
        for pr in range(2):
            jp = 2 * qi + pr
            first = st["n"] == 0
            last = st["n"] == JP - 1
            st["n"] += 1
            for co in range(CCH):
                nc.tensor.matmul(
                    pools["attp"][co // 2][:, co % 2, :],
                    vt[jp][:, :, co * 128:(co + 1) * 128],
                    eT[:, 2 * pr:2 * pr + 2, :], start=first, stop=last,
                    perf_mode=DR)
            nc.tensor.matmul(pools["den"][:, :], ones8[:, :, :],
                             eT[:, 2 * pr:2 * pr + 2, :], start=first,
                             stop=last, perf_mode=DR)

    def att_feed(ci, qis, st, depth=1):
        for qi in qis:
            st["q"].append((qi, scores_quad(ci, qi)))
            while len(st["q"]) > depth:
                qi0, eT0 = st["q"].popleft()
                attnv_quad(qi0, eT0, st)

    def att_flush(st):
        while st["q"]:
            qi0, eT0 = st["q"].popleft()
            attnv_quad(qi0, eT0, st)

    def den_rec():
        dsb = pio.tile([128, IBW], F32R, tag="dsb", name="dsb", bufs=2)
        nc.vector.tensor_copy(dsb[:, :], pools["den"][:, :])
        rec = pio.tile([128, 4], F32, tag="rec", name="rec", bufs=2)
        if DBG_STUB_REC:
            nc.vector.memset(rec, 0.001)
            return rec
        dT = pools["mm"].tile([128, 512], F32, tag="mm", name="dT")
        for it in range(2):
            nc.tensor.matmul(dT[:, 2 * it:2 * it + 2],
                             dsb[:, it * 128:(it + 1) * 128],
                             e1r[:, 0:2], start=True, stop=True,
                             skip_group_check=True)
        nc.vector.reciprocal_approx_fast(out=rec[:, 0:4], in_=dT[:, 0:4])
        return rec

    def ao_cast(ci):
        ao = [pio.tile([128, 2, IBW], FP8, tag=f"ao{t}", name=f"ao{t}",
                       bufs=4) for t in range(NT)]
        for t in range(NT):
            if AO_ENG[ci][t] == "a":
                nc.scalar.activation(ao[t][:, :, :], pools["attp"][t][:, :, :],
                                     AF.Copy, bias=0.0, scale=S_AO)
            else:
                nc.vector.tensor_scalar(out=ao[t][:, :, :],
                                        in0=pools["attp"][t][:, :, :],
                                        scalar1=S_AO, scalar2=None,
                                        op0=OP.mult)
        return ao

    def finalize(ci, rec, ao):
        for it in range(2):
            rows = slice(ci * IBW + it * 128, ci * IBW + (it + 1) * 128)
            pp = pools["mm"].tile([128, 512], F32, tag="mm", name="mm")
            for t in range(NT):
                nc.tensor.matmul(
                    pp[:, :], ao[t][:, :, it * 128:(it + 1) * 128],
                    w8["wo8"][:, 2 * t:2 * t + 2, :], start=(t == 0),
                    stop=(t == NT - 1), perf_mode=DR)
            fin = pio.tile([128, 512], F32, tag="fin", name="fin", bufs=3)
            nc.vector.scalar_tensor_tensor(
                out=fin[:, :], in0=pp[:, :], scalar=rec[:, 2 * it:2 * it + 1],
                in1=xr[ci * 2 + it][:, :], op0=OP.mult, op1=OP.add)
            nc.sync.dma_start(out=dr["y"][rows, :], in_=fin[:, :])

    # ---- emission schedule ----
    # call 0 (i 0:256) is fused into KV production, lagging ATT_LAG blocks so
    # the wide kv casts drain while PE runs attention on older blocks
    # ---- scope A: KV + Q production with a deep (bufs=KV_BUFS) quad
    # rotation filling all of PSUM ----
    with tc.tile_pool(name="pkv", bufs=KV_BUFS, space="PSUM") as pkv:
        pools["pq"] = pkv
        # dummy matmuls: anchor the PE p-state ramp during the DMA fill so
        # real matmuls run at full clock from the start
        wps = pquad_tile("warmmm")
        for i in range(64):
            nc.tensor.matmul(wps[:, 0, 0:128], ones8[:, :, :], ones8[:, :, :],
                             start=(i == 0), stop=(i == 63), perf_mode=DR)
        kv(0)
        qprod(0)
        kv(1)
        qprod(1)
        for jb in range(2, JB):
            kv(jb)

    # ---- scope B: attention + projection ----
    with tc.tile_pool(name="pquad", bufs=2, space="PSUM") as pquad, \
         tc.tile_pool(name="patt", bufs=1, space="PSUM") as patt, \
         tc.tile_pool(name="pmm", bufs=1, space="PSUM") as pmm:
        pools["pq"] = pquad
        pools["mm"] = pmm
        pools["attp"] = [
            patt.tile([128, 2, IBW], F32, tag=f"att{t}", name=f"att{t}")
            for t in range(NT)]
        pools["den"] = patt.tile([128, IBW], F32, tag="den", name="den")
        st0 = {"q": deque(), "n": 0}
        att_feed(0, range(0, JB), st0)
        att_flush(st0)
        recs = {0: den_rec()}
        aos = {0: ao_cast(0)}
        for ci in range(1, NCALL):
            st = {"q": deque(), "n": 0}
            att_feed(ci, range(0, 3), st)
            finalize(ci - 1, recs[ci - 1], aos[ci - 1])
            att_feed(ci, range(3, JB), st)
            att_flush(st)
            recs[ci] = den_rec()
            aos[ci] = ao_cast(ci)
        finalize(NCALL - 1, recs[NCALL - 1], aos[NCALL - 1])


_NC_CACHE = {}


def _get_nc(reps: int = 1):
    if reps not in _NC_CACHE:
        _NC_CACHE[reps] = build(reps)
    return _NC_CACHE[reps]


def _pack_w(wt):
    """[Cout, Cin] (already scaled) -> [128, CCH, Cout] fp8, [p, ci, co] =
    wt[co, ci*128+p]."""
    return np.ascontiguousarray(
        wt.T.reshape(CCH, 128, C).transpose(1, 0, 2)).astype(E4)


def _host_inputs(x, norm_gamma, norm_beta, wq, bq, wk, bk, wv, bv, wo, bo):
    f32, f64 = np.float32, np.float64
    x = np.asarray(x, f32)
    gamma = np.asarray(norm_gamma, f64)
    beta = np.asarray(norm_beta, f64)
    wqd, bqd = np.asarray(wq, f64), np.asarray(bq, f64)
    wkd = np.asarray(wk, f64)
    wvd, bvd = np.asarray(wv, f64), np.asarray(bv, f64)
    wod, bod = np.asarray(wo, f64), np.asarray(bo, f64)

    in_maps = []
    per_batch = {}
    for b in range(B):
        xb = x[b].reshape(C, HW)
        xg = xb.astype(f64).reshape(NG, (C // NG) * HW)
        mean, var = xg.mean(axis=1), xg.var(axis=1)
        rstd = 1.0 / np.sqrt(var + EPS)
        gmat = gamma.reshape(NG, C // NG)
        A = (gmat * rstd[:, None]).reshape(C)
        Bv = (beta.reshape(NG, C // NG) - (mean * rstd)[:, None] * gmat
              ).reshape(C)
        tqv = WS * (bqd + wqd @ Bv)
        bo2 = bod + wod @ (bvd + wvd @ Bv)
        per_batch[b] = dict(
            xb=xb,
            wq8=_pack_w(WS * wqd * A[None, :]),
            wk8=_pack_w(WS * wkd * A[None, :]),
            wv8=_pack_w(WS * wvd * A[None, :]),
            wo8=_pack_w(WS * wod),
            tqb=np.ascontiguousarray(tqv.reshape(CCH, 128).T, dtype=f32),
            bo2=bo2,
        )
    for core in range(NCORES):
        b, s = core // NSLICE, core % NSLICE
        pb = per_batch[b]
        xb = pb["xb"]
        xl = np.roll(xb, -s * SL, axis=1)  # local j order: queries at cols 0..SL
        x8 = np.ascontiguousarray(
            xl.reshape(CCH, 128, HW).transpose(1, 0, 2)).astype(E4)
        xrT = np.ascontiguousarray(
            xb[:, s * SL:(s + 1) * SL].T.astype(f64) + pb["bo2"][None, :], f32)
        in_maps.append(dict(x8=x8, wq8=pb["wq8"], wk8=pb["wk8"],
                            wv8=pb["wv8"], wo8=pb["wo8"], tqb=pb["tqb"],
                            xrT=xrT))
    return in_maps


def kernel(x, norm_gamma, norm_beta, wq, bq, wk, bk, wv, bv, wo, bo,
           reps: int = 1):
    nc = _get_nc(reps)
    in_maps = _host_inputs(x, norm_gamma, norm_beta, wq, bq, wk, bk, wv, bv,
                           wo, bo)
    res = run_bass_kernel_spmd(nc, in_maps, core_ids=list(range(NCORES)),
                               trace=False)
    out = np.empty((B, C, HW), np.float32)
    for core in range(NCORES):
        b, s = core // NSLICE, core % NSLICE
        out[b][:, s * SL:(s + 1) * SL] = res.results[core]["y"].T
    return out.reshape(B, C, H, W)


# revision 40
# speedup vs baseline: 2.9248x; 1.0679x over previous
"""AttnBlock (GroupNorm -> QKV -> 4096x4096 spatial attention -> proj -> residual)
for Trainium2, sharded over 8 NeuronCores. fp8e4m3 DoubleRow edition.

Sharding: core = (batch b, query-slice s); b = core//4, s = core%4. Each core
computes K/V for its full batch image (redundant across the 4 cores of a
batch) and attention/projection for its 1024-query slice. No collectives.

Host-side prep (exact, in f64/f32):
  - GroupNorm is folded into the weights: A = gamma*rstd, B = beta - mean*A
    (per batch); wq' = 8*(wq . diag(A)) etc., so the device consumes raw x.
  - x is quantized to fp8 on host (2MB/core instead of 8MB), and its columns
    are rotated by the core's query-slice offset so the SPMD program always
    reads its queries from local columns [0, 1024) (softmax is j-order
    invariant, so K/V order doesn't matter).
  - The K bias drops entirely: softmax(q.(k+bk)) == softmax(q.k + const_j).
  - All weights are pre-scaled by 8 to center fp8 quantization; the exp is
    shifted by -2 (softmax-invariant) to keep e^s inside fp8 range; both
    factors cancel exactly through the final reciprocal/projection scaling.

Device math (all matmuls fp8e4m3 with MatmulPerfMode.DoubleRow: K=256 per
instruction at 0.5 cycles/row = 4x the f32r rate):
  k = wk'@x8, v = wv'@x8, q = wq'@x8 + tq  (KV psums packed in pairs inside
  [128,4,256] quad banks -> one 1024-col fp8 cast per pair on ACT/DVE)
  eT[j,i] = fp8(exp(k^T q * scale - 2))    (one ACT exp per 4-chunk quad)
  att[c,i] += v eT ; den[i] += 1^T eT      (PSUM accumulation over all j)
  out = (wo'@(att/512)) * (8/den) + (x^T + bo2)   (proj + fused residual)
Queries processed in four 256-wide i-blocks; the first is fused into KV
production, the rest stream afterwards. K/V/x resident in SBUF as fp8.
"""
import numpy as np
import ml_dtypes
import concourse.bacc as bacc
import concourse.bass as bass
import concourse.tile as tile
import concourse.mybir as mybir
from concourse.bass_utils import run_bass_kernel_spmd

F32 = mybir.dt.float32
F32R = mybir.dt.float32r
FP8 = mybir.dt.float8e4
AF = mybir.ActivationFunctionType
OP = mybir.AluOpType
DR = mybir.MatmulPerfMode.DoubleRow
E4 = ml_dtypes.float8_e4m3

B, C, H, W = 2, 512, 64, 64
HW = H * W                    # 4096
NCORES = 8
NSLICE = 4                    # query slices per batch
SL = HW // NSLICE             # 1024 query positions per core
NG = 32                       # groups
EPS = 1e-6
CCH = C // 128                # 4 channel chunks
NT = CCH // 2                 # 2 chunk-pairs per C contraction (DoubleRow)
JB = HW // 512                # 8 j-blocks
JC = HW // 128                # 32 j-chunks
JP = JC // 2                  # 16 j-pairs of 256
NCALL = 4                     # i-blocks of 256 per core
IBW = SL // NCALL             # 256
WS = 8.0                      # host weight prescale
SHIFT = -2.0                  # exp bias (softmax-invariant)
S_AO = 1.0 / 512              # att -> fp8 cast scale
E1V = 0.125                   # rec transpose scale -> rec = 8/den
SC2 = float(C) ** -0.5 / (WS * WS)
DBG_STUB_REC = False
# engine maps ("a"=ACT, "v"=DVE); Pool/gpsimd cannot read PSUM on trn2.
# KV_ENG: engine of the four wide kv casts per jb (qk1, qk2, qv1, qv2)
KV_ENG = {}
KV_ENG_LATE = "vava"
AO_ENG = {0: "vv", 1: "vv", 2: "vv", 3: "av"}
ATT_LAG = 3
FUSE0 = True
KV_SPLIT = False  # split each wide kv cast across both engines
KV_BUFS = 4       # scope-A quad rotation depth
ATT_DEPTH = 3     # attnv lag in quads
ET_BUFS = 4
WARM_N = 64


def build(reps: int = 1):
    nc = bacc.Bacc("TRN2", target_bir_lowering=False)
    dr = {}
    dr["x8"] = nc.dram_tensor("x8", [128, CCH, HW], FP8, kind="ExternalInput")
    for w in ("wq8", "wk8", "wv8", "wo8"):
        dr[w] = nc.dram_tensor(w, [128, CCH, C], FP8, kind="ExternalInput")
    dr["tqb"] = nc.dram_tensor("tqb", [128, CCH], F32, kind="ExternalInput")
    dr["xrT"] = nc.dram_tensor("xrT", [SL, C], F32, kind="ExternalInput")
    dr["y"] = nc.dram_tensor("y", [SL, C], F32, kind="ExternalOutput")
    with tile.TileContext(nc) as tc:
        _body(nc, tc, reps, dr)
    nc.finalize()
    return nc


def _body(nc, tc, reps, dr):
    from contextlib import ExitStack
    with ExitStack() as ctx:
        pc = ctx.enter_context(tc.tile_pool(name="pc", bufs=1))
        pio = ctx.enter_context(tc.tile_pool(name="pio", bufs=1))

        # small consts
        ones8 = pc.tile([128, 2, 128], FP8, tag="ones8", name="ones8")
        nc.vector.memset(ones8, 1.0)
        e1f = pc.tile([128, 2], F32, tag="e1f", name="e1f")
        nc.vector.memset(e1f, 0.0)
        nc.vector.memset(e1f[0:1, 0:2], E1V)
        e1r = pc.tile([128, 2], F32R, tag="e1r", name="e1r")
        nc.vector.tensor_copy(e1r[:, :], e1f[:, :])
        bsh = pc.tile([128, 1], F32, tag="bsh", name="bsh")
        nc.vector.memset(bsh, SHIFT)
        warm = pc.tile([128, 1], F32, tag="warm", name="warm")
        nc.scalar.activation(warm[:, :], bsh[:, 0:1], AF.Exp)

        tq = pc.tile([128, CCH], F32, tag="tq", name="tq")

        # persistent fp8 operands
        x8 = pc.tile([128, CCH, HW], FP8, tag="x8", name="x8")
        k8 = pc.tile([128, CCH, HW], FP8, tag="k8", name="k8")
        q8 = pc.tile([128, CCH, SL], FP8, tag="q8", name="q8")
        w8 = {}
        for w in ("wq8", "wk8", "wv8", "wo8"):
            w8[w] = pc.tile([128, CCH, C], FP8, tag=w, name=w)
        vt = [pc.tile([128, 2, 512], FP8, tag=f"vt{j}", name=f"vt{j}")
              for j in range(JP)]
        xr = [pc.tile([128, C], F32, tag=f"xr{i}", name=f"xr{i}")
              for i in range(8)]

        # input DMAs in consumption order
        def dma_x8_cols(c0, c1):
            nc.sync.dma_start(
                out=x8[:, :, c0:c1],
                in_=bass.AP(tensor=dr["x8"], offset=c0,
                            ap=[[CCH * HW, 128], [HW, CCH], [1, c1 - c0]]))

        def dma_w(name):
            nc.sync.dma_start(
                out=w8[name],
                in_=bass.AP(tensor=dr[name], offset=0,
                            ap=[[CCH * C, 128], [C, CCH], [1, C]]))

        dma_x8_cols(0, 512)
        dma_w("wk8")
        dma_w("wv8")
        dma_x8_cols(512, 1024)
        dma_w("wq8")
        nc.sync.dma_start(out=tq, in_=dr["tqb"][:, :])
        for c in range(1, 4):
            dma_x8_cols(c * 1024, (c + 1) * 1024)
        dma_w("wo8")
        for i in range(8):
            nc.sync.dma_start(
                out=xr[i], in_=dr["xrT"][i * 128:(i + 1) * 128, :])

        cst = dict(ones8=ones8, e1r=e1r, bsh=bsh, tq=tq, x8=x8, k8=k8, q8=q8,
                   w8=w8, vt=vt, xr=xr)
        for _ in range(reps):
            _attn_once(nc, tc, pc, pio, dr, cst)


def _attn_once(nc, tc, pc, pio, dr, cst):
    from collections import deque
    x8, k8, q8, w8 = cst["x8"], cst["k8"], cst["q8"], cst["w8"]
    vt, xr = cst["vt"], cst["xr"]
    ones8, e1r, bsh, tq = cst["ones8"], cst["e1r"], cst["bsh"], cst["tq"]
    pools = {}

    def pquad_tile(name="qd"):
        return pools["pq"].tile([128, 4, 256], F32, tag="qd", name=name)

    def cast(eng, out, in_):
        if eng == "a":
            nc.scalar.copy(out, in_)
        else:
            nc.vector.tensor_copy(out, in_)

    def kv(jb):
        engs = KV_ENG.get(jb, KV_ENG_LATE)
        jsl = slice(jb * 512, (jb + 1) * 512)
        for h in range(2):  # K co-pairs (0,1) and (2,3)
            qk = pquad_tile()
            for g in range(2):
                co = 2 * h + g
                for t in range(NT):
                    nc.tensor.matmul(
                        qk[:, 2 * g:2 * g + 2, :],
                        w8["wk8"][:, 2 * t:2 * t + 2, co * 128:(co + 1) * 128],
                        x8[:, 2 * t:2 * t + 2, jsl], start=(t == 0),
                        stop=(t == NT - 1), perf_mode=DR)
            if KV_SPLIT:
                e0 = engs[h]
                e1 = "a" if e0 == "v" else "v"
                cast(e0, k8[:, 2 * h, jsl], qk[:, 0:2, :])
                cast(e1, k8[:, 2 * h + 1, jsl], qk[:, 2:4, :])
            else:
                cast(engs[h], k8[:, 2 * h:2 * h + 2, jsl], qk[:, :, :])
        for h in range(2):  # V jt-pairs -> vt tiles
            qv = pquad_tile()
            for g in range(2):
                jt = 2 * h + g
                j0 = jb * 512 + jt * 128
                for t in range(NT):
                    nc.tensor.matmul(
                        qv[:, 2 * g:2 * g + 2, :],
                        x8[:, 2 * t:2 * t + 2, j0:j0 + 128],
                        w8["wv8"][:, 2 * t:2 * t + 2, :], start=(t == 0),
                        stop=(t == NT - 1), perf_mode=DR)
            if KV_SPLIT:
                e0 = engs[2 + h]
                e1 = "a" if e0 == "v" else "v"
                cast(e0, vt[2 * jb + h][:, 0, :], qv[:, 0:2, :])
                cast(e1, vt[2 * jb + h][:, 1, :], qv[:, 2:4, :])
            else:
                cast(engs[2 + h], vt[2 * jb + h][:, :, :], qv[:, :, :])

    def qprod(ib):
        isl = slice(ib * 512, (ib + 1) * 512)
        for h in range(2):
            qq = pquad_tile()
            for g in range(2):
                co = 2 * h + g
                for t in range(NT):
                    nc.tensor.matmul(
                        qq[:, 2 * g:2 * g + 2, :],
                        w8["wq8"][:, 2 * t:2 * t + 2, co * 128:(co + 1) * 128],
                        x8[:, 2 * t:2 * t + 2, isl], start=(t == 0),
                        stop=(t == NT - 1), perf_mode=DR)
                nc.scalar.activation(q8[:, co, isl], qq[:, 2 * g:2 * g + 2, :],
                                     AF.Identity, bias=tq[:, co:co + 1],
                                     scale=1.0)

    def scores_quad(ci, qi):
        isl = slice(ci * IBW, (ci + 1) * IBW)
        sq = pquad_tile()
        for qj in range(4):
            jc = 4 * qi + qj
            for t in range(NT):
                nc.tensor.matmul(
                    sq[:, qj, :], k8[:, 2 * t:2 * t + 2, jc * 128:(jc + 1) * 128],
                    q8[:, 2 * t:2 * t + 2, isl], start=(t == 0),
                    stop=(t == NT - 1), perf_mode=DR)
        eT = pio.tile([128, 4, 256], FP8, tag="eT", name="eT", bufs=ET_BUFS)
        nc.scalar.activation(eT[:, :, :], sq[:, :, :], AF.Exp,
                             bias=bsh[:, 0:1], scale=SC2)
        return eT

    def attnv_quad(qi, eT, st):
        for pr in range(2):
            jp = 2 * qi + pr
            first = st["n"] == 0
            last = st["n"] == JP - 1
            st["n"] += 1
            for co in range(CCH):
                nc.tensor.matmul(
                    pools["attp"][co // 2][:, co % 2, :],
                    vt[jp][:, :, co * 128:(co + 1) * 128],
                    eT[:, 2 * pr:2 * pr + 2, :], start=first, stop=last,
                    perf_mode=DR)
            nc.tensor.matmul(pools["den"][:, :], ones8[:, :, :],
                             eT[:, 2 * pr:2 * pr + 2, :], start=first,
                             stop=last, perf_mode=DR)

    def att_feed(ci, qis, st, depth=ATT_DEPTH):
        for qi in qis:
            st["q"].append((qi, scores_quad(ci, qi)))
            while len(st["q"]) > depth:
                qi0, eT0 = st["q"].popleft()
                attnv_quad(qi0, eT0, st)

    def att_flush(st):
        while st["q"]:
            qi0, eT0 = st["q"].popleft()
            attnv_quad(qi0, eT0, st)

    def den_rec():
        dsb = pio.tile([128, IBW], F32R, tag="dsb", name="dsb", bufs=2)
        nc.vector.tensor_copy(dsb[:, :], pools["den"][:, :])
        rec = pio.tile([128, 4], F32, tag="rec", name="rec", bufs=2)
        if DBG_STUB_REC:
            nc.vector.memset(rec, 0.001)
            return rec
        dT = pools["mm"].tile([128, 512], F32, tag="mm", name="dT")
        for it in range(2):
            nc.tensor.matmul(dT[:, 2 * it:2 * it + 2],
                             dsb[:, it * 128:(it + 1) * 128],
                             e1r[:, 0:2], start=True, stop=True,
                             skip_group_check=True)
        nc.vector.reciprocal_approx_fast(out=rec[:, 0:4], in_=dT[:, 0:4])
        return rec

    def ao_cast(ci):
        ao = [pio.tile([128, 2, IBW], FP8, tag=f"ao{t}", name=f"ao{t}",
                       bufs=4) for t in range(NT)]
        for t in range(NT):
            if AO_ENG[ci][t] == "a":
                nc.scalar.activation(ao[t][:, :, :], pools["attp"][t][:, :, :],
                                     AF.Copy, bias=0.0, scale=S_AO)
            else:
                nc.vector.tensor_scalar(out=ao[t][:, :, :],
                                        in0=pools["attp"][t][:, :, :],
                                        scalar1=S_AO, scalar2=None,
                                        op0=OP.mult)
        return ao

    def finalize(ci, rec, ao):
        for it in range(2):
            rows = slice(ci * IBW + it * 128, ci * IBW + (it + 1) * 128)
            pp = pools["mm"].tile([128, 512], F32, tag="mm", name="mm")
            for t in range(NT):
                nc.tensor.matmul(
                    pp[:, :], ao[t][:, :, it * 128:(it + 1) * 128],
                    w8["wo8"][:, 2 * t:2 * t + 2, :], start=(t == 0),
                    stop=(t == NT - 1), perf_mode=DR)
            fin = pio.tile([128, 512], F32, tag="fin", name="fin", bufs=3)
            nc.vector.scalar_tensor_tensor(
                out=fin[:, :], in0=pp[:, :], scalar=rec[:, 2 * it:2 * it + 1],
                in1=xr[ci * 2 + it][:, :], op0=OP.mult, op1=OP.add)
            nc.sync.dma_start(out=dr["y"][rows, :], in_=fin[:, :])

    # ---- emission schedule ----
    # call 0 (i 0:256) is fused into KV production, lagging ATT_LAG blocks so
    # the wide kv casts drain while PE runs attention on older blocks
    # ---- scope A: KV + Q production with a deep (bufs=KV_BUFS) quad
    # rotation filling all of PSUM ----
    with tc.tile_pool(name="pkv", bufs=KV_BUFS, space="PSUM") as pkv:
        pools["pq"] = pkv
        # dummy matmuls: anchor the PE p-state ramp during the DMA fill so
        # real matmuls run at full clock from the start
        wps = pquad_tile("warmmm")
        for i in range(WARM_N):
            nc.tensor.matmul(wps[:, 0, 0:128], ones8[:, :, :], ones8[:, :, :],
                             start=(i == 0), stop=(i == WARM_N - 1), perf_mode=DR)
        kv(0)
        qprod(0)
        kv(1)
        qprod(1)
        for jb in range(2, JB):
            kv(jb)

    # ---- scope B: attention + projection ----
    with tc.tile_pool(name="pquad", bufs=2, space="PSUM") as pquad, \
         tc.tile_pool(name="patt", bufs=1, space="PSUM") as patt, \
         tc.tile_pool(name="pmm", bufs=1, space="PSUM") as pmm:
        pools["pq"] = pquad
        pools["mm"] = pmm
        pools["attp"] = [
            patt.tile([128, 2, IBW], F32, tag=f"att{t}", name=f"att{t}")
            for t in range(NT)]
        pools["den"] = patt.tile([128, IBW], F32, tag="den", name="den")
        st0 = {"q": deque(), "n": 0}
        att_feed(0, range(0, JB), st0)
        att_flush(st0)
        recs = {0: den_rec()}
        aos = {0: ao_cast(0)}
        for ci in range(1, NCALL):
            st = {"q": deque(), "n": 0}
            att_feed(ci, range(0, 3), st)
            finalize(ci - 1, recs[ci - 1], aos[ci - 1])
            att_feed(ci, range(3, JB), st)
            att_flush(st)
            recs[ci] = den_rec()
            aos[ci] = ao_cast(ci)
        finalize(NCALL - 1, recs[NCALL - 1], aos[NCALL - 1])


_NC_CACHE = {}


def _get_nc(reps: int = 1):
    if reps not in _NC_CACHE:
        _NC_CACHE[reps] = build(reps)
    return _NC_CACHE[reps]


def _pack_w(wt):
    """[Cout, Cin] (already scaled) -> [128, CCH, Cout] fp8, [p, ci, co] =
    wt[co, ci*128+p]."""
    return np.ascontiguousarray(
        wt.T.reshape(CCH, 128, C).transpose(1, 0, 2)).astype(E4)


def _host_inputs(x, norm_gamma, norm_beta, wq, bq, wk, bk, wv, bv, wo, bo):
    f32, f64 = np.float32, np.float64
    x = np.asarray(x, f32)
    gamma = np.asarray(norm_gamma, f64)
    beta = np.asarray(norm_beta, f64)
    wqd, bqd = np.asarray(wq, f64), np.asarray(bq, f64)
    wkd = np.asarray(wk, f64)
    wvd, bvd = np.asarray(wv, f64), np.asarray(bv, f64)
    wod, bod = np.asarray(wo, f64), np.asarray(bo, f64)

    in_maps = []
    per_batch = {}
    for b in range(B):
        xb = x[b].reshape(C, HW)
        xg = xb.astype(f64).reshape(NG, (C // NG) * HW)
        mean, var = xg.mean(axis=1), xg.var(axis=1)
        rstd = 1.0 / np.sqrt(var + EPS)
        gmat = gamma.reshape(NG, C // NG)
        A = (gmat * rstd[:, None]).reshape(C)
        Bv = (beta.reshape(NG, C // NG) - (mean * rstd)[:, None] * gmat
              ).reshape(C)
        tqv = WS * (bqd + wqd @ Bv)
        bo2 = bod + wod @ (bvd + wvd @ Bv)
        per_batch[b] = dict(
            xb=xb,
            wq8=_pack_w(WS * wqd * A[None, :]),
            wk8=_pack_w(WS * wkd * A[None, :]),
            wv8=_pack_w(WS * wvd * A[None, :]),
            wo8=_pack_w(WS * wod),
            tqb=np.ascontiguousarray(tqv.reshape(CCH, 128).T, dtype=f32),
            bo2=bo2,
        )
    for core in range(NCORES):
        b, s = core // NSLICE, core % NSLICE
        pb = per_batch[b]
        xb = pb["xb"]
        xl = np.roll(xb, -s * SL, axis=1)  # local j order: queries at cols 0..SL
        x8 = np.ascontiguousarray(
            xl.reshape(CCH, 128, HW).transpose(1, 0, 2)).astype(E4)
        xrT = np.ascontiguousarray(
            xb[:, s * SL:(s + 1) * SL].T.astype(f64) + pb["bo2"][None, :], f32)
        in_maps.append(dict(x8=x8, wq8=pb["wq8"], wk8=pb["wk8"],
                            wv8=pb["wv8"], wo8=pb["wo8"], tqb=pb["tqb"],
                            xrT=xrT))
    return in_maps


def kernel(x, norm_gamma, norm_beta, wq, bq, wk, bk, wv, bv, wo, bo,
           reps: int = 1):
    nc = _get_nc(reps)
    in_maps = _host_inputs(x, norm_gamma, norm_beta, wq, bq, wk, bk, wv, bv,
                           wo, bo)
    res = run_bass_kernel_spmd(nc, in_maps, core_ids=list(range(NCORES)),
                               trace=False)
    out = np.empty((B, C, HW), np.float32)
    for core in range(NCORES):
        b, s = core // NSLICE, core % NSLICE
        out[b][:, s * SL:(s + 1) * SL] = res.results[core]["y"].T
    return out.reshape(B, C, H, W)
